# revision 35
# baseline (speedup 1.0000x reference)
"""Multi-head attention (RoPE, causal) on 8 TRN2 NeuronCores.

Sharding: DP2 x TP4. Core c handles batch b = c//4 and heads
H_c = {4*(c%4) .. 4*(c%4)+3}. Attention outputs are exchanged with two
8-rank AllToAlls (bf16, q-sliced), after which every core computes the
final out-projection for a 256-row q-slice of BOTH batches with the full
head dimension locally. No reduction collective; the host-side unshard
is a pure concatenation.

Single fused device phase per head-pair: the QKV projection + RoPE for
s-chunk sc+1 is emitted as PE "filler" work interleaved into the
attention k-block loop of q-chunk sc, so the ACT-bound softmax stretches
keep the PE array busy (and HAM-warm). Scores for the two heads of a
pair run concurrently in the top/bottom 64-row halves of the PE array
(row tiling via base-partition-derived tile_position); their exps are
batched into one [128,1024] ACT op spanning two PSUM banks. Softmax
denominators ride a ones-column in the V weights; normalization uses
reciprocal_approx_fast and is deferred into the next chunk so the PE
never waits on the DVE. Elementwise copies/masks ride on GPSIMD.
"""

import sys
from collections import deque

for _p in ("/opt/trn_rl_repo",):
    if _p not in sys.path:
        sys.path.insert(0, _p)

import numpy as np
import ml_dtypes

from concourse import bacc, bass, mybir, tile
from concourse.bass_utils import run_bass_kernel_spmd

F32 = mybir.dt.float32
BF16 = mybir.dt.bfloat16

D, H, HD, S, B = 1024, 16, 64, 2048, 2
HPC = 4          # heads per core
NP = 2           # head pairs per core
QC = 512         # q-chunk size
KB = 128         # k-block size
NQC = S // QC    # 4
NKB = S // KB    # 16
NC = 8           # total cores; the AllToAll spans all 8
SLC = S // NC    # 256 rows of final output per core (for BOTH batches)

Ident = mybir.ActivationFunctionType.Identity
Exp = mybir.ActivationFunctionType.Exp
Add = mybir.AluOpType.add

# feature flags (sim/HW divergence bisection)
BATCH_EXP = False   # one [128,1024] ACT spanning both heads' PSUM banks
RECIP_FAST = True  # reciprocal_approx_fast instead of nc.vector.reciprocal
TS_BIAS = True     # DVE tensor_scalar bias-add instead of ACT Identity
GP_TRI = False      # causal fine-triangle mask on GPSIMD instead of DVE


def _host_constants():
    pos = np.arange(S, dtype=np.float64)
    inv_freq = 1.0 / (10000.0 ** (np.arange(0, HD, 2, dtype=np.float64) / HD))
    freqs = np.outer(pos, inv_freq)
    cosT = np.repeat(np.cos(freqs), 2, axis=1).T.astype(np.float32)  # [64, S]
    sinT = np.repeat(np.sin(freqs), 2, axis=1).T.astype(np.float32)
    # pair-stacked: same table on both 64-partition halves
    cosT = np.concatenate([cosT, cosT], axis=0)  # [128, S]
    sinT = np.concatenate([sinT, sinT], axis=0)
    perm = np.zeros((128, 128), dtype=np.float32)
    for base in (0, 64):
        for i in range(32):
            perm[base + 2 * i + 1, base + 2 * i] = -1.0
            perm[base + 2 * i, base + 2 * i + 1] = 1.0
    # causal fine triangle for a 128-col diagonal slice: keep q >= k
    tri = (np.arange(128)[None, :] >= np.arange(128)[:, None]).astype(np.float32)
    return cosT, sinT, perm, tri


def build_program():
    cosT, sinT, perm_np, tri_np = _host_constants()

    nc = bacc.Bacc(None, target_bir_lowering=False)

    # --- I/O ---------------------------------------------------------
    xb = nc.declare_dram_parameter("xb", [D, S], BF16, isOutput=False)
    wq = nc.declare_dram_parameter("wq", [D, 256], BF16, isOutput=False)
    wk = nc.declare_dram_parameter("wk", [D, 256], BF16, isOutput=False)
    wv = nc.declare_dram_parameter("wv", [D, 260], BF16, isOutput=False)
    bq = nc.declare_dram_parameter("bq", [NP, 128], F32, isOutput=False)
    bk = nc.declare_dram_parameter("bk", [NP, 128], F32, isOutput=False)
    bv = nc.declare_dram_parameter("bv", [1, 260], BF16, isOutput=False)
    ones = nc.declare_dram_parameter("ones", [128, 128], BF16, isOutput=False)
    perm = nc.declare_dram_parameter("perm", [128, 128], BF16, isOutput=False)
    wout = nc.declare_dram_parameter("wout", [D, D], BF16, isOutput=False)
    bout = nc.declare_dram_parameter("bout", [1, D], BF16, isOutput=False)
    out = nc.declare_dram_parameter("out_s", [B, SLC, D], F32, isOutput=True)

    # bf16 constants as two inline tensors: the small one (perm+tri)
    # unblocks the warmup matmuls immediately; cos/sin follow the Q/K
    # weights since RoPE runs after the first projections.
    csa_np = np.concatenate([perm_np, tri_np], axis=1).astype(ml_dtypes.bfloat16)
    csb_np = np.concatenate([cosT, sinT], axis=1).astype(ml_dtypes.bfloat16)
    csa_c = nc.inline_tensor(csa_np, name="csa_c")
    csb_c = nc.inline_tensor(csb_np, name="csb_c")

    with tile.TileContext(nc) as tc:
        with (
            tc.tile_pool(name="persist", bufs=1) as pp,
            tc.tile_pool(name="dram", bufs=1, space="DRAM") as dp,
        ):
            # --- constants / weights into SBUF ------------------------
            # DMA issue has a ~2us per-op serial cost on each HWDGE ring,
            # so everything is loaded with FEW, BIG DMAs: x^T (transposed
            # on the host) in 4 s-window chunks on the sync ring; one DMA
            # per weight tensor + one merged-constants DMA on the ACT
            # ring.
            csa_s = pp.tile([128, 256], BF16)
            nc.scalar.dma_start(out=csa_s[:], in_=csa_c[:])
            perm_s = csa_s[:, 0:128]
            tri_s = csa_s[:, 128:256]

            # x^T arrives pre-transposed from the host as xb=[D, S];
            # load per s-window so the first projection starts early.
            # xt layout: [128, 8*S], d-chunk dc at cols [S*dc, +S).
            xt = pp.tile([128, 8 * S], BF16)
            for sc in range(NQC):
                nc.sync.dma_start(
                    out=xt.rearrange("p (c s) -> p c s", c=8)[
                        :, :, QC * sc : QC * sc + QC
                    ],
                    in_=xb.rearrange("(c p) s -> p c s", p=128)[
                        :, :, QC * sc : QC * sc + QC
                    ],
                )

            wk_s = pp.tile([128, 8, 256], BF16)
            wv_s = pp.tile([128, 8, 260], BF16)
            wq_s = pp.tile([128, 8, 256], BF16)
            nc.scalar.dma_start(out=wk_s[:], in_=wk.rearrange("(c p) n -> p c n", p=128))
            nc.scalar.dma_start(out=wv_s[:], in_=wv.rearrange("(c p) n -> p c n", p=128))
            csb_s = pp.tile([128, 2 * S], BF16)
            nc.scalar.dma_start(out=csb_s[:], in_=csb_c[:])
            cos_s = csb_s[:, 0:S]
            sin_s = csb_s[:, S : 2 * S]
            nc.scalar.dma_start(out=wq_s[:], in_=wq.rearrange("(c p) n -> p c n", p=128))
            ones_f = pp.tile([128, 128], BF16)
            nc.scalar.dma_start(out=ones_f[:], in_=ones[:])
            bq_s = pp.tile([128, NP], F32)
            bk_s = pp.tile([128, NP], F32)
            bv_s = pp.tile([1, 260], BF16)
            nc.scalar.dma_start(out=bv_s[:], in_=bv[:])
            nc.scalar.dma_start(out=bq_s[:], in_=bq.rearrange("p n -> n p"))
            nc.scalar.dma_start(out=bk_s[:], in_=bk.rearrange("p n -> n p"))
            # out-projection weights ride the sync ring behind the x^T
            # loads: done by ~30us, well before the out-projection, and
            # ahead of the first cc_in write on that ring.
            wo_s = pp.tile([128, 8, D], BF16)
            bo_s = pp.tile([1, D], BF16)
            nc.sync.dma_start(out=bo_s[:], in_=bout[:])
            nc.sync.dma_start(
                out=wo_s[:], in_=wout.rearrange("(c p) n -> p c n", p=128)
            )

            # persistent activations
            qt = pp.tile([128, NP * S], BF16)   # rotated Q^T, pair-major
            kt = pp.tile([128, NP * S], BF16)   # rotated K^T
            # vt[p][kb]: [128, 2*65] — per-pair V (+ ones column per head)
            vt = [
                [pp.tile([128, 2 * 65], BF16, name=f"vt{p}_{i}") for i in range(NKB)]
                for p in range(NP)
            ]
            # attnT[p]: [64, 2*S] — within-pair head h at cols [S*h, S*(h+1))
            attnT = [pp.tile([64, NP * S], BF16, name=f"attnT{p}") for p in range(NP)]

            # DRAM bounce buffers for the per-pair 8-rank AllToAll
            cc_in = [
                dp.tile([NC, 128, SLC], BF16, name=f"cc_in{p}") for p in range(NP)
            ]
            cc_out = [
                dp.tile([NC, 128, SLC], BF16, name=f"cc_out{p}") for p in range(NP)
            ]
            # gathered attn^T: af[b2][k] = head-dim chunk k for batch b2
            af = [
                [pp.tile([128, SLC], BF16, name=f"af{b2}_{k}") for k in range(8)]
                for b2 in range(B)
            ]

            # =============================================================
            # Fused phase: per pair, QKV projection + RoPE interleaved into
            # the attention k-block loop as PE filler work.
            # =============================================================
            with (
                tc.tile_pool(name="sb_work", bufs=4) as sbw,
                tc.tile_pool(name="p_pool", bufs=3) as ppool,
                tc.tile_pool(name="avs_pool", bufs=3) as avsp,
                tc.tile_pool(name="rc_pool", bufs=2) as rcp,
                tc.tile_pool(name="sc_psum", bufs=2, space="PSUM") as scp,
                tc.tile_pool(name="av_psum", bufs=2, space="PSUM") as avp,
                tc.tile_pool(name="wk_psum", bufs=2, space="PSUM") as wkp,
            ):
                # HAM warmup: tiny matmuls so the PE activity window is
                # already busy when the real projection stream arrives.
                wu = wkp.tile([128, 512], F32, tag="work", name="wu")
                for i in range(150):
                    nc.tensor.matmul(
                        wu[0:1, 0:64],
                        perm_s[0:1, 0:1],
                        perm_s[0:1, 0:64],
                        start=True,
                        stop=True,
                    )

                # ---- filler units: projection + RoPE ------------------
                def emit_proj_qk(p, sc, w_s, b_s, rot):
                    ssl = slice(QC * sc, QC * sc + QC)
                    ps = wkp.tile([128, 512], F32, tag="work", name="pj")
                    for c in range(8):
                        nc.tensor.matmul(
                            ps[:],
                            w_s[:, c, 128 * p : 128 * p + 128],
                            xt[:, S * c + QC * sc : S * c + QC * sc + QC],
                            start=(c == 0),
                            stop=(c == 7),
                        )
                    raw = sbw.tile([128, 512], BF16, tag="raw")
                    if TS_BIAS:
                        nc.vector.tensor_scalar(
                            raw[:], ps[:], b_s[:, p : p + 1], None, op0=Add
                        )
                    else:
                        nc.scalar.activation(
                            raw[:], ps[:], Ident, bias=b_s[:, p : p + 1]
                        )
                    pr = wkp.tile([128, 512], F32, tag="work", name="pr")
                    nc.tensor.matmul(pr[:], perm_s[:], raw[:], start=True, stop=True)
                    dst = rot[:, S * p + QC * sc : S * p + QC * sc + QC]
                    rtmp = sbw.tile([128, 512], BF16, tag="rtmp")
                    nc.vector.tensor_mul(dst, raw[:], cos_s[:, ssl])
                    nc.vector.tensor_mul(rtmp[:], pr[:], sin_s[:, ssl])
                    nc.vector.tensor_add(dst, dst, rtmp[:])

                def emit_proj_v(p, sb):
                    ps = wkp.tile([128, 512], F32, tag="work", name="pv")
                    for c in range(8):
                        nc.tensor.matmul(
                            ps[:, 0:130],
                            xt[:, S * c + 128 * sb : S * c + 128 * sb + 128],
                            wv_s[:, c, 130 * p : 130 * p + 130],
                            start=(c == 0),
                            stop=False,
                        )
                    nc.tensor.matmul(
                        ps[:, 0:130],
                        ones_f[0:1, 0:128],
                        bv_s[:, 130 * p : 130 * p + 130],
                        start=False,
                        stop=True,
                    )
                    nc.vector.tensor_copy(vt[p][sb][:], ps[:, 0:130])

                def stage_proj(fill, p, sc):
                    # K and V before Q: the next q-chunk's k-loop touches
                    # them first.
                    fill.append(
                        (9, lambda p=p, sc=sc: emit_proj_qk(p, sc, wk_s, bk_s, kt))
                    )
                    for sb in range(4 * sc, 4 * sc + 4):
                        fill.append((3, lambda p=p, sb=sb: emit_proj_v(p, sb)))
                    fill.append(
                        (9, lambda p=p, sc=sc: emit_proj_qk(p, sc, wq_s, bq_s, qt))
                    )

                def pop_fill(fill, budget):
                    while budget > 0 and fill:
                        c, fn = fill.popleft()
                        fn()
                        budget -= c
                    return budget

                # ---- attention chunk ---------------------------------
                def emit_scores(p, qc, kb):
                    mrel = kb - 4 * qc
                    c0 = 128 * max(mrel, 0)
                    sc2 = scp.tile([128, 1024], F32, tag="sc", name="sc2")
                    for h in range(2):
                        hsl = slice(64 * h, 64 * h + 64)
                        nc.tensor.matmul(
                            sc2[:, 512 * h + c0 : 512 * h + 512],
                            kt[hsl, S * p + KB * kb : S * p + KB * kb + KB],
                            qt[hsl, S * p + QC * qc + c0 : S * p + QC * qc + 512],
                            start=True,
                            stop=True,
                        )
                    p2 = ppool.tile([128, 1024], BF16, tag="p2")
                    if c0 == 0 and BATCH_EXP:
                        # both heads' exps in one ACT op (the matmuls wrote
                        # the full [0:1024] span of this PSUM tile).
                        nc.scalar.activation(
                            p2[:], sc2[:], Exp, scale=float(HD**-0.5)
                        )
                    else:
                        for h in range(2):
                            nc.scalar.activation(
                                p2[:, 512 * h + c0 : 512 * h + 512],
                                sc2[:, 512 * h + c0 : 512 * h + 512],
                                Exp,
                                scale=float(HD**-0.5),
                            )
                    if mrel >= 0:
                        eng = nc.gpsimd if GP_TRI else nc.vector
                        for h in range(2):
                            eng.tensor_mul(
                                p2[:, 512 * h + c0 : 512 * h + c0 + 128],
                                p2[:, 512 * h + c0 : 512 * h + c0 + 128],
                                tri_s[:],
                            )
                    return p2, c0

                def emit_av(p, qc, kb, av, p2c):
                    p2, c0 = p2c
                    nkb_q = 4 * qc + 4
                    for h in range(2):
                        nc.tensor.matmul(
                            av[h][0:65, c0:512],
                            vt[p][kb][:, 65 * h : 65 * h + 65],
                            p2[:, 512 * h + c0 : 512 * h + 512],
                            start=(kb == 0),
                            stop=(kb == nkb_q - 1),
                        )

                def stage_norm(p, qc, av):
                    # emit avs/denominator copies now (frees the av PSUM
                    # banks); the reciprocal chain is returned for
                    # deferred emission. The denominator row is copied to
                    # a partition-0 tile: custom-DVE ops must not read at
                    # a nonzero base partition.
                    avs, dns = [], []
                    for h in range(2):
                        a = avsp.tile([64, 512], F32, tag="avs", name="avs")
                        nc.vector.tensor_copy(a[:], av[h][0:64, :])
                        d = rcp.tile([1, 512], F32, tag="dn", name="dn")
                        nc.vector.tensor_copy(d[:], av[h][64:65, :])
                        avs.append(a)
                        dns.append(d)

                    def finish(p=p, qc=qc, avs=avs, dns=dns):
                        for h in range(2):
                            rcb = rcp.tile([1, 512], BF16, tag="rcb", name="rcb")
                            if RECIP_FAST:
                                rcf = rcp.tile([1, 512], F32, tag="rcf", name="rcf")
                                nc.vector.reciprocal_approx_fast(
                                    rcf[:], dns[h][:]
                                )
                                nc.vector.tensor_copy(rcb[:], rcf[:])
                            else:
                                with nc.allow_low_precision(
                                    reason="softmax denom reciprocal; bf16 ok"
                                ):
                                    nc.vector.reciprocal(rcb[:], dns[h][:])
                            bc = scp.tile([64, 512], F32, tag="sc", name="bc")
                            nc.tensor.matmul(
                                bc[:],
                                ones_f[0:1, 0:64],
                                rcb[:],
                                start=True,
                                stop=True,
                            )
                            nc.vector.tensor_mul(
                                attnT[p][
                                    :, S * h + QC * qc : S * h + QC * qc + QC
                                ],
                                avs[h][:],
                                bc[:],
                            )
                            # stream this chunk's slice of the exchange
                            # payload out right away: 1:1 with the mul
                            # that produced it. Sync ring only: DMAs on
                            # the ACT ring would block later ACTIVATEs.
                            nc.sync.dma_start(
                                out=cc_in[p].rearrange("g p q -> p g q")[
                                    64 * h : 64 * h + 64, 2 * qc : 2 * qc + 2
                                ],
                                in_=attnT[p][
                                    :, S * h + QC * qc : S * h + QC * qc + QC
                                ].rearrange("p (g q) -> p g q", g=2),
                            )

                    return finish

                def emit_attn_chunk(p, qc, fill, pending_norm):
                    nkb_q = 4 * qc + 4
                    av = [
                        avp.tile([65, 512], F32, tag="av", name=f"av{_h}")
                        for _h in range(2)
                    ]
                    fill_per_kb = (
                        sum(c for c, _ in fill) + nkb_q - 1
                    ) // nkb_q if fill else 0
                    pipe = []
                    for kb in range(nkb_q):
                        pipe.append((kb, emit_scores(p, qc, kb)))
                        if kb == 2 and pending_norm is not None:
                            pending_norm()
                            pending_norm = None
                        pop_fill(fill, fill_per_kb)
                        if len(pipe) > 2:
                            kb0, p2c = pipe.pop(0)
                            emit_av(p, qc, kb0, av, p2c)
                    if pending_norm is not None:
                        pending_norm()
                    for kb0, p2c in pipe:
                        emit_av(p, qc, kb0, av, p2c)
                    return stage_norm(p, qc, av)

                # ---- per-pair streams --------------------------------
                for p in range(NP):
                    fill = deque()
                    if p == 0:
                        stage_proj(fill, 0, 0)
                        pop_fill(fill, 10**9)  # qc0 needs all of sc0
                    pending_norm = None
                    for qc in range(NQC):
                        if qc < NQC - 1:
                            stage_proj(fill, p, qc + 1)
                        elif p == 0:
                            stage_proj(fill, 1, 0)  # prefetch pair 1
                        pending_norm = emit_attn_chunk(p, qc, fill, pending_norm)
                        if qc == NQC - 1:
                            pop_fill(fill, 10**9)
                            pending_norm()
                    # exchange this pair's attention output (cc_in slices
                    # were streamed out per chunk by the norm stages)
                    nc.gpsimd.collective_compute(
                        "AllToAll",
                        mybir.AluOpType.bypass,
                        ins=[cc_in[p].opt()],
                        outs=[cc_out[p].opt()],
                        replica_groups=[[0, 1, 2, 3, 4, 5, 6, 7]],
                    )
                    # gathered tiles: source core 4*b2 + g, pair p ->
                    # head-dim chunk k = 2g + p of batch b2.
                    for src in range(NC):
                        b2, g = src // 4, src % 4
                        nc.sync.dma_start(
                            out=af[b2][2 * g + p][:], in_=cc_out[p][src]
                        )

            # =============================================================
            # Out-projection for my q-slice. The pair-0 half of every
            # slot's accumulation runs during the second AllToAll.
            # =============================================================
            with (
                tc.tile_pool(name="out_sb", bufs=4) as osp,
                tc.tile_pool(name="op_psum", bufs=8, space="PSUM") as opp,
            ):
                slots = []  # (psum, b2, sb, nsl)
                for b2 in range(B):
                    for sb in range(SLC // 128):
                        for nc2 in range(2):
                            nsl = slice(512 * nc2, 512 * nc2 + 512)
                            ps = opp.tile(
                                [128, 512], F32, tag="op", name=f"op{b2}{sb}{nc2}"
                            )
                            slots.append((ps, b2, sb, nsl))

                for ps, b2, sb, nsl in slots:
                    nc.tensor.matmul(
                        ps[:],
                        ones_f[0:1, 0:128],
                        bo_s[:, nsl],
                        start=True,
                        stop=False,
                    )
                    for k in range(0, 8, 2):
                        nc.tensor.matmul(
                            ps[:],
                            af[b2][k][:, 128 * sb : 128 * sb + 128],
                            wo_s[:, k, nsl],
                            start=False,
                            stop=False,
                        )
                o_t = None
                for i, (ps, b2, sb, nsl) in enumerate(slots):
                    for k in range(1, 8, 2):
                        nc.tensor.matmul(
                            ps[:],
                            af[b2][k][:, 128 * sb : 128 * sb + 128],
                            wo_s[:, k, nsl],
                            start=False,
                            stop=(k == 7),
                        )
                    # stage both 512-col halves of a (b2, sb) row block
                    # into one tile and ship it with a single DMA.
                    if i % 2 == 0:
                        o_t = osp.tile([128, D], F32, tag="o")
                    nc.vector.tensor_copy(o_t[:, nsl], ps[:])
                    if i % 2 == 1:
                        eng = nc.sync if i % 4 == 1 else nc.scalar
                        eng.dma_start(
                            out=out[b2, 128 * sb : 128 * sb + 128, :], in_=o_t[:]
                        )
    nc.finalize()
    return nc


_PROGRAM = None


def _get_program():
    global _PROGRAM
    if _PROGRAM is None:
        _PROGRAM = build_program()
    return _PROGRAM


def make_in_maps(x, Wqkv, bqkv, Wout, bout):
    x = np.asarray(x, dtype=np.float32)
    Wqkv = np.asarray(Wqkv, dtype=np.float32)
    bqkv = np.asarray(bqkv, dtype=np.float32)
    Wout = np.asarray(Wout, dtype=np.float32)
    bout = np.asarray(bout, dtype=np.float32)

    wout_bf = Wout.astype(ml_dtypes.bfloat16)
    bout_bf = bout.reshape(1, D).astype(ml_dtypes.bfloat16)
    _, _, perm_np, _ = _host_constants()
    ones_np = np.ones((128, 128), dtype=ml_dtypes.bfloat16)
    in_maps = []
    for c in range(8):
        b, g = c // 4, c % 4
        cols = slice(64 * HPC * g, 64 * HPC * (g + 1))  # this core's head dims
        # V weights augmented with a zero column per head slot; the matching
        # bias element is 1.0, so V tiles come out as [v(64) | 1] per head.
        wv_aug = np.zeros((D, 65 * HPC), dtype=np.float32)
        bv_aug = np.zeros((1, 65 * HPC), dtype=np.float32)
        wv_c = Wqkv[:, 2 * D :][:, cols]
        bv_c = bqkv[2 * D :][cols]
        for h in range(HPC):
            wv_aug[:, 65 * h : 65 * h + 64] = wv_c[:, 64 * h : 64 * h + 64]
            bv_aug[0, 65 * h : 65 * h + 64] = bv_c[64 * h : 64 * h + 64]
            bv_aug[0, 65 * h + 64] = 1.0
        in_maps.append(
            {
                "xb": np.ascontiguousarray(x[:, b, :].T).astype(ml_dtypes.bfloat16),
                "wq": np.ascontiguousarray(Wqkv[:, 0 * D :][:, cols]).astype(
                    ml_dtypes.bfloat16
                ),
                "wk": np.ascontiguousarray(Wqkv[:, 1 * D :][:, cols]).astype(
                    ml_dtypes.bfloat16
                ),
                "wv": wv_aug.astype(ml_dtypes.bfloat16),
                "bq": np.ascontiguousarray(bqkv[0 * D :][cols].reshape(NP, 128)),
                "bk": np.ascontiguousarray(bqkv[1 * D :][cols].reshape(NP, 128)),
                "bv": bv_aug.astype(ml_dtypes.bfloat16),
                "ones": ones_np,
                "perm": perm_np.astype(ml_dtypes.bfloat16),
                "wout": wout_bf,
                "bout": bout_bf,
            }
        )
    return in_maps


def unshard(results):
    out = np.empty((S, B, D), dtype=np.float32)
    for r in range(8):
        for b2 in range(B):
            out[SLC * r : SLC * (r + 1), b2, :] = results[r]["out_s"][b2]
    return out


def kernel(x, Wqkv, bqkv, Wout, bout, **_kw):
    nc = _get_program()
    in_maps = make_in_maps(x, Wqkv, bqkv, Wout, bout)
    res = run_bass_kernel_spmd(nc, in_maps, list(range(8)))
    return unshard(res.results)


# revision 36
# speedup vs baseline: 1.3473x; 1.3473x over previous
"""Multi-head attention (RoPE, causal) on 8 TRN2 NeuronCores.

Sharding: DP2 x TP4. Core c handles batch b = c//4 and heads
H_c = {4*(c%4) .. 4*(c%4)+3}. Attention outputs are exchanged with two
8-rank AllToAlls (bf16, q-sliced), after which every core computes the
final out-projection for a 256-row q-slice of BOTH batches with the full
head dimension locally. No reduction collective; the host-side unshard
is a pure concatenation.

Single fused device phase per head-pair: the QKV projection + RoPE for
s-chunk sc+1 is emitted as PE "filler" work interleaved into the
attention k-block loop of q-chunk sc, so the ACT-bound softmax stretches
keep the PE array busy (and HAM-warm). Scores for the two heads of a
pair run concurrently in the top/bottom 64-row halves of the PE array
(row tiling via base-partition-derived tile_position); their exps are
batched into one [128,1024] ACT op spanning two PSUM banks. Softmax
denominators ride a ones-column in the V weights; normalization uses
reciprocal_approx_fast and is deferred into the next chunk so the PE
never waits on the DVE. Elementwise copies/masks ride on GPSIMD.
"""

import sys
from collections import deque

for _p in ("/opt/trn_rl_repo",):
    if _p not in sys.path:
        sys.path.insert(0, _p)

import numpy as np
import ml_dtypes

from concourse import bacc, bass, mybir, tile
from concourse.bass_utils import run_bass_kernel_spmd

F32 = mybir.dt.float32
BF16 = mybir.dt.bfloat16

D, H, HD, S, B = 1024, 16, 64, 2048, 2
HPC = 4          # heads per core
NP = 2           # head pairs per core
QC = 512         # q-chunk size
KB = 128         # k-block size
NQC = S // QC    # 4
NKB = S // KB    # 16
NC = 8           # total cores; the AllToAll spans all 8
SLC = S // NC    # 256 rows of final output per core (for BOTH batches)

Ident = mybir.ActivationFunctionType.Identity
Exp = mybir.ActivationFunctionType.Exp
Add = mybir.AluOpType.add

# feature flags (sim/HW divergence bisection)
BATCH_EXP = True   # one [128,1024] ACT spanning both heads' PSUM banks
RECIP_FAST = True  # reciprocal_approx_fast instead of nc.vector.reciprocal
TS_BIAS = True     # DVE tensor_scalar bias-add instead of ACT Identity
GP_TRI = False      # causal fine-triangle mask on GPSIMD instead of DVE


def _host_constants():
    pos = np.arange(S, dtype=np.float64)
    inv_freq = 1.0 / (10000.0 ** (np.arange(0, HD, 2, dtype=np.float64) / HD))
    freqs = np.outer(pos, inv_freq)
    cosT = np.repeat(np.cos(freqs), 2, axis=1).T.astype(np.float32)  # [64, S]
    sinT = np.repeat(np.sin(freqs), 2, axis=1).T.astype(np.float32)
    # pair-stacked: same table on both 64-partition halves
    cosT = np.concatenate([cosT, cosT], axis=0)  # [128, S]
    sinT = np.concatenate([sinT, sinT], axis=0)
    perm = np.zeros((128, 128), dtype=np.float32)
    for base in (0, 64):
        for i in range(32):
            perm[base + 2 * i + 1, base + 2 * i] = -1.0
            perm[base + 2 * i, base + 2 * i + 1] = 1.0
    # causal fine triangle for a 128-col diagonal slice: keep q >= k
    tri = (np.arange(128)[None, :] >= np.arange(128)[:, None]).astype(np.float32)
    return cosT, sinT, perm, tri


def build_program():
    cosT, sinT, perm_np, tri_np = _host_constants()

    nc = bacc.Bacc(None, target_bir_lowering=False)

    # --- I/O ---------------------------------------------------------
    xb = nc.declare_dram_parameter("xb", [D, S], BF16, isOutput=False)
    wq = nc.declare_dram_parameter("wq", [D, 256], BF16, isOutput=False)
    wk = nc.declare_dram_parameter("wk", [D, 256], BF16, isOutput=False)
    wv = nc.declare_dram_parameter("wv", [D, 260], BF16, isOutput=False)
    bq = nc.declare_dram_parameter("bq", [NP, 128], F32, isOutput=False)
    bk = nc.declare_dram_parameter("bk", [NP, 128], F32, isOutput=False)
    bv = nc.declare_dram_parameter("bv", [1, 260], BF16, isOutput=False)
    ones = nc.declare_dram_parameter("ones", [128, 128], BF16, isOutput=False)
    perm = nc.declare_dram_parameter("perm", [128, 128], BF16, isOutput=False)
    wout = nc.declare_dram_parameter("wout", [D, D], BF16, isOutput=False)
    bout = nc.declare_dram_parameter("bout", [1, D], BF16, isOutput=False)
    out = nc.declare_dram_parameter("out_s", [B, SLC, D], F32, isOutput=True)

    # bf16 constants as two inline tensors: the small one (perm+tri)
    # unblocks the warmup matmuls immediately; cos/sin follow the Q/K
    # weights since RoPE runs after the first projections.
    csa_np = np.concatenate([perm_np, tri_np], axis=1).astype(ml_dtypes.bfloat16)
    csb_np = np.concatenate([cosT, sinT], axis=1).astype(ml_dtypes.bfloat16)
    csa_c = nc.inline_tensor(csa_np, name="csa_c")
    csb_c = nc.inline_tensor(csb_np, name="csb_c")

    with tile.TileContext(nc) as tc:
        with (
            tc.tile_pool(name="persist", bufs=1) as pp,
            tc.tile_pool(name="dram", bufs=1, space="DRAM") as dp,
        ):
            # --- constants / weights into SBUF ------------------------
            # DMA issue has a ~2us per-op serial cost on each HWDGE ring,
            # so everything is loaded with FEW, BIG DMAs: x^T (transposed
            # on the host) in 4 s-window chunks on the sync ring; one DMA
            # per weight tensor + one merged-constants DMA on the ACT
            # ring.
            csa_s = pp.tile([128, 256], BF16)
            nc.scalar.dma_start(out=csa_s[:], in_=csa_c[:])
            perm_s = csa_s[:, 0:128]
            tri_s = csa_s[:, 128:256]

            # x^T arrives pre-transposed from the host as xb=[D, S];
            # load per s-window so the first projection starts early.
            # xt layout: [128, 8*S], d-chunk dc at cols [S*dc, +S).
            xt = pp.tile([128, 8 * S], BF16)
            for sc in range(NQC):
                nc.sync.dma_start(
                    out=xt.rearrange("p (c s) -> p c s", c=8)[
                        :, :, QC * sc : QC * sc + QC
                    ],
                    in_=xb.rearrange("(c p) s -> p c s", p=128)[
                        :, :, QC * sc : QC * sc + QC
                    ],
                )

            wk_s = pp.tile([128, 8, 256], BF16)
            wv_s = pp.tile([128, 8, 260], BF16)
            wq_s = pp.tile([128, 8, 256], BF16)
            nc.scalar.dma_start(out=wk_s[:], in_=wk.rearrange("(c p) n -> p c n", p=128))
            nc.scalar.dma_start(out=wv_s[:], in_=wv.rearrange("(c p) n -> p c n", p=128))
            csb_s = pp.tile([128, 2 * S], BF16)
            nc.scalar.dma_start(out=csb_s[:], in_=csb_c[:])
            cos_s = csb_s[:, 0:S]
            sin_s = csb_s[:, S : 2 * S]
            nc.scalar.dma_start(out=wq_s[:], in_=wq.rearrange("(c p) n -> p c n", p=128))
            ones_f = pp.tile([128, 128], BF16)
            nc.scalar.dma_start(out=ones_f[:], in_=ones[:])
            bq_s = pp.tile([128, NP], F32)
            bk_s = pp.tile([128, NP], F32)
            bv_s = pp.tile([1, 260], BF16)
            nc.scalar.dma_start(out=bv_s[:], in_=bv[:])
            nc.scalar.dma_start(out=bq_s[:], in_=bq.rearrange("p n -> n p"))
            nc.scalar.dma_start(out=bk_s[:], in_=bk.rearrange("p n -> n p"))
            # out-projection weights ride the sync ring behind the x^T
            # loads: done by ~30us, well before the out-projection, and
            # ahead of the first cc_in write on that ring.
            wo_s = pp.tile([128, 8, D], BF16)
            bo_s = pp.tile([1, D], BF16)
            nc.sync.dma_start(out=bo_s[:], in_=bout[:])
            nc.sync.dma_start(
                out=wo_s[:], in_=wout.rearrange("(c p) n -> p c n", p=128)
            )

            # persistent activations
            qt = pp.tile([128, NP * S], BF16)   # rotated Q^T, pair-major
            kt = pp.tile([128, NP * S], BF16)   # rotated K^T
            # vt[p][kb]: [128, 2*65] — per-pair V (+ ones column per head)
            vt = [
                [pp.tile([128, 2 * 65], BF16, name=f"vt{p}_{i}") for i in range(NKB)]
                for p in range(NP)
            ]
            # attnT[p]: [64, 2*S] — within-pair head h at cols [S*h, S*(h+1))
            attnT = [pp.tile([64, NP * S], BF16, name=f"attnT{p}") for p in range(NP)]

            # DRAM bounce buffers for the per-pair 8-rank AllToAll
            cc_in = [
                dp.tile([NC, 128, SLC], BF16, name=f"cc_in{p}") for p in range(NP)
            ]
            cc_out = [
                dp.tile([NC, 128, SLC], BF16, name=f"cc_out{p}") for p in range(NP)
            ]
            # gathered attn^T: af[b2][k] = head-dim chunk k for batch b2
            af = [
                [pp.tile([128, SLC], BF16, name=f"af{b2}_{k}") for k in range(8)]
                for b2 in range(B)
            ]

            # =============================================================
            # Fused phase: per pair, QKV projection + RoPE interleaved into
            # the attention k-block loop as PE filler work.
            # =============================================================
            with (
                tc.tile_pool(name="sb_work", bufs=4) as sbw,
                tc.tile_pool(name="p_pool", bufs=3) as ppool,
                tc.tile_pool(name="avs_pool", bufs=3) as avsp,
                tc.tile_pool(name="rc_pool", bufs=2) as rcp,
                tc.tile_pool(name="sc_psum", bufs=2, space="PSUM") as scp,
                tc.tile_pool(name="av_psum", bufs=2, space="PSUM") as avp,
                tc.tile_pool(name="wk_psum", bufs=2, space="PSUM") as wkp,
            ):
                # HAM warmup: tiny matmuls so the PE activity window is
                # already busy when the real projection stream arrives.
                wu = wkp.tile([128, 512], F32, tag="work", name="wu")
                for i in range(24):
                    nc.tensor.matmul(
                        wu[0:1, 0:64],
                        perm_s[0:1, 0:1],
                        perm_s[0:1, 0:64],
                        start=True,
                        stop=True,
                    )

                # ---- filler units: projection + RoPE ------------------
                def emit_proj_qk(p, sc, w_s, b_s, rot):
                    ssl = slice(QC * sc, QC * sc + QC)
                    ps = wkp.tile([128, 512], F32, tag="work", name="pj")
                    for c in range(8):
                        nc.tensor.matmul(
                            ps[:],
                            w_s[:, c, 128 * p : 128 * p + 128],
                            xt[:, S * c + QC * sc : S * c + QC * sc + QC],
                            start=(c == 0),
                            stop=(c == 7),
                        )
                    raw = sbw.tile([128, 512], BF16, tag="raw")
                    if TS_BIAS:
                        nc.vector.tensor_scalar(
                            raw[:], ps[:], b_s[:, p : p + 1], None, op0=Add
                        )
                    else:
                        nc.scalar.activation(
                            raw[:], ps[:], Ident, bias=b_s[:, p : p + 1]
                        )
                    pr = wkp.tile([128, 512], F32, tag="work", name="pr")
                    nc.tensor.matmul(pr[:], perm_s[:], raw[:], start=True, stop=True)
                    dst = rot[:, S * p + QC * sc : S * p + QC * sc + QC]
                    rtmp = sbw.tile([128, 512], BF16, tag="rtmp")
                    nc.vector.tensor_mul(dst, raw[:], cos_s[:, ssl])
                    nc.vector.tensor_mul(rtmp[:], pr[:], sin_s[:, ssl])
                    nc.vector.tensor_add(dst, dst, rtmp[:])

                def emit_proj_v(p, sb):
                    ps = wkp.tile([128, 512], F32, tag="work", name="pv")
                    for c in range(8):
                        nc.tensor.matmul(
                            ps[:, 0:130],
                            xt[:, S * c + 128 * sb : S * c + 128 * sb + 128],
                            wv_s[:, c, 130 * p : 130 * p + 130],
                            start=(c == 0),
                            stop=False,
                        )
                    nc.tensor.matmul(
                        ps[:, 0:130],
                        ones_f[0:1, 0:128],
                        bv_s[:, 130 * p : 130 * p + 130],
                        start=False,
                        stop=True,
                    )
                    nc.vector.tensor_copy(vt[p][sb][:], ps[:, 0:130])

                def stage_proj(fill, p, sc):
                    # K and V before Q: the next q-chunk's k-loop touches
                    # them first.
                    fill.append(
                        (9, lambda p=p, sc=sc: emit_proj_qk(p, sc, wk_s, bk_s, kt))
                    )
                    for sb in range(4 * sc, 4 * sc + 4):
                        fill.append((3, lambda p=p, sb=sb: emit_proj_v(p, sb)))
                    fill.append(
                        (9, lambda p=p, sc=sc: emit_proj_qk(p, sc, wq_s, bq_s, qt))
                    )

                def pop_fill(fill, budget):
                    while budget > 0 and fill:
                        c, fn = fill.popleft()
                        fn()
                        budget -= c
                    return budget

                # ---- attention chunk ---------------------------------
                def emit_scores(p, qc, kb):
                    mrel = kb - 4 * qc
                    c0 = 128 * max(mrel, 0)
                    sc2 = scp.tile([128, 1024], F32, tag="sc", name="sc2")
                    for h in range(2):
                        hsl = slice(64 * h, 64 * h + 64)
                        nc.tensor.matmul(
                            sc2[:, 512 * h + c0 : 512 * h + 512],
                            kt[hsl, S * p + KB * kb : S * p + KB * kb + KB],
                            qt[hsl, S * p + QC * qc + c0 : S * p + QC * qc + 512],
                            start=True,
                            stop=True,
                        )
                    p2 = ppool.tile([128, 1024], BF16, tag="p2")
                    if c0 == 0 and BATCH_EXP:
                        # both heads' exps in one ACT op (the matmuls wrote
                        # the full [0:1024] span of this PSUM tile).
                        nc.scalar.activation(
                            p2[:], sc2[:], Exp, scale=float(HD**-0.5)
                        )
                    else:
                        for h in range(2):
                            nc.scalar.activation(
                                p2[:, 512 * h + c0 : 512 * h + 512],
                                sc2[:, 512 * h + c0 : 512 * h + 512],
                                Exp,
                                scale=float(HD**-0.5),
                            )
                    if mrel >= 0:
                        eng = nc.gpsimd if GP_TRI else nc.vector
                        for h in range(2):
                            eng.tensor_mul(
                                p2[:, 512 * h + c0 : 512 * h + c0 + 128],
                                p2[:, 512 * h + c0 : 512 * h + c0 + 128],
                                tri_s[:],
                            )
                    return p2, c0

                def emit_av(p, qc, kb, av, p2c):
                    p2, c0 = p2c
                    nkb_q = 4 * qc + 4
                    for h in range(2):
                        nc.tensor.matmul(
                            av[h][0:65, c0:512],
                            vt[p][kb][:, 65 * h : 65 * h + 65],
                            p2[:, 512 * h + c0 : 512 * h + 512],
                            start=(kb == 0),
                            stop=(kb == nkb_q - 1),
                        )

                def stage_norm(p, qc, av):
                    # emit avs/denominator copies now (frees the av PSUM
                    # banks); the reciprocal chain is returned for
                    # deferred emission. The denominator row is copied to
                    # a partition-0 tile: custom-DVE ops must not read at
                    # a nonzero base partition.
                    avs, dns = [], []
                    for h in range(2):
                        a = avsp.tile([64, 512], F32, tag="avs", name="avs")
                        nc.vector.tensor_copy(a[:], av[h][0:64, :])
                        d = rcp.tile([1, 512], F32, tag="dn", name="dn")
                        nc.vector.tensor_copy(d[:], av[h][64:65, :])
                        avs.append(a)
                        dns.append(d)

                    def finish(p=p, qc=qc, avs=avs, dns=dns):
                        for h in range(2):
                            rcb = rcp.tile([1, 512], BF16, tag="rcb", name="rcb")
                            if RECIP_FAST:
                                rcf = rcp.tile([1, 512], F32, tag="rcf", name="rcf")
                                nc.vector.reciprocal_approx_fast(
                                    rcf[:], dns[h][:]
                                )
                                nc.vector.tensor_copy(rcb[:], rcf[:])
                            else:
                                with nc.allow_low_precision(
                                    reason="softmax denom reciprocal; bf16 ok"
                                ):
                                    nc.vector.reciprocal(rcb[:], dns[h][:])
                            bc = scp.tile([64, 512], F32, tag="sc", name="bc")
                            nc.tensor.matmul(
                                bc[:],
                                ones_f[0:1, 0:64],
                                rcb[:],
                                start=True,
                                stop=True,
                            )
                            nc.vector.tensor_mul(
                                attnT[p][
                                    :, S * h + QC * qc : S * h + QC * qc + QC
                                ],
                                avs[h][:],
                                bc[:],
                            )
                            # stream this chunk's slice of the exchange
                            # payload out right away: 1:1 with the mul
                            # that produced it. Sync ring only: DMAs on
                            # the ACT ring would block later ACTIVATEs.
                            nc.sync.dma_start(
                                out=cc_in[p].rearrange("g p q -> p g q")[
                                    64 * h : 64 * h + 64, 2 * qc : 2 * qc + 2
                                ],
                                in_=attnT[p][
                                    :, S * h + QC * qc : S * h + QC * qc + QC
                                ].rearrange("p (g q) -> p g q", g=2),
                            )

                    return finish

                def emit_attn_chunk(p, qc, fill, pending_norm):
                    nkb_q = 4 * qc + 4
                    av = [
                        avp.tile([65, 512], F32, tag="av", name=f"av{_h}")
                        for _h in range(2)
                    ]
                    fill_per_kb = (
                        sum(c for c, _ in fill) + nkb_q - 1
                    ) // nkb_q if fill else 0
                    pipe = []
                    for kb in range(nkb_q):
                        pipe.append((kb, emit_scores(p, qc, kb)))
                        if kb == 2 and pending_norm is not None:
                            pending_norm()
                            pending_norm = None
                        pop_fill(fill, fill_per_kb)
                        if len(pipe) > 2:
                            kb0, p2c = pipe.pop(0)
                            emit_av(p, qc, kb0, av, p2c)
                    if pending_norm is not None:
                        pending_norm()
                    for kb0, p2c in pipe:
                        emit_av(p, qc, kb0, av, p2c)
                    return stage_norm(p, qc, av)

                # ---- per-pair streams --------------------------------
                for p in range(NP):
                    fill = deque()
                    if p == 0:
                        stage_proj(fill, 0, 0)
                        pop_fill(fill, 10**9)  # qc0 needs all of sc0
                    pending_norm = None
                    for qc in range(NQC):
                        if qc < NQC - 1:
                            stage_proj(fill, p, qc + 1)
                        elif p == 0:
                            stage_proj(fill, 1, 0)  # prefetch pair 1
                        pending_norm = emit_attn_chunk(p, qc, fill, pending_norm)
                        if qc == NQC - 1:
                            pop_fill(fill, 10**9)
                            pending_norm()
                    # exchange this pair's attention output (cc_in slices
                    # were streamed out per chunk by the norm stages)
                    nc.gpsimd.collective_compute(
                        "AllToAll",
                        mybir.AluOpType.bypass,
                        ins=[cc_in[p].opt()],
                        outs=[cc_out[p].opt()],
                        replica_groups=[[0, 1, 2, 3, 4, 5, 6, 7]],
                    )
                    # gathered tiles: source core 4*b2 + g, pair p ->
                    # head-dim chunk k = 2g + p of batch b2.
                    for src in range(NC):
                        b2, g = src // 4, src % 4
                        nc.sync.dma_start(
                            out=af[b2][2 * g + p][:], in_=cc_out[p][src]
                        )

            # =============================================================
            # Out-projection for my q-slice. The pair-0 half of every
            # slot's accumulation runs during the second AllToAll.
            # =============================================================
            with (
                tc.tile_pool(name="out_sb", bufs=4) as osp,
                tc.tile_pool(name="op_psum", bufs=8, space="PSUM") as opp,
            ):
                slots = []  # (psum, b2, sb, nsl)
                for b2 in range(B):
                    for sb in range(SLC // 128):
                        for nc2 in range(2):
                            nsl = slice(512 * nc2, 512 * nc2 + 512)
                            ps = opp.tile(
                                [128, 512], F32, tag="op", name=f"op{b2}{sb}{nc2}"
                            )
                            slots.append((ps, b2, sb, nsl))

                for ps, b2, sb, nsl in slots:
                    nc.tensor.matmul(
                        ps[:],
                        ones_f[0:1, 0:128],
                        bo_s[:, nsl],
                        start=True,
                        stop=False,
                    )
                    for k in range(0, 8, 2):
                        nc.tensor.matmul(
                            ps[:],
                            af[b2][k][:, 128 * sb : 128 * sb + 128],
                            wo_s[:, k, nsl],
                            start=False,
                            stop=False,
                        )
                o_t = None
                for i, (ps, b2, sb, nsl) in enumerate(slots):
                    for k in range(1, 8, 2):
                        nc.tensor.matmul(
                            ps[:],
                            af[b2][k][:, 128 * sb : 128 * sb + 128],
                            wo_s[:, k, nsl],
                            start=False,
                            stop=(k == 7),
                        )
                    # stage both 512-col halves of a (b2, sb) row block
                    # into one tile and ship it with a single DMA.
                    if i % 2 == 0:
                        o_t = osp.tile([128, D], F32, tag="o")
                    nc.vector.tensor_copy(o_t[:, nsl], ps[:])
                    if i % 2 == 1:
                        eng = nc.sync if i % 4 == 1 else nc.scalar
                        eng.dma_start(
                            out=out[b2, 128 * sb : 128 * sb + 128, :], in_=o_t[:]
                        )
    nc.finalize()
    return nc


_PROGRAM = None


def _get_program():
    global _PROGRAM
    if _PROGRAM is None:
        _PROGRAM = build_program()
    return _PROGRAM


def make_in_maps(x, Wqkv, bqkv, Wout, bout):
    x = np.asarray(x, dtype=np.float32)
    Wqkv = np.asarray(Wqkv, dtype=np.float32)
    bqkv = np.asarray(bqkv, dtype=np.float32)
    Wout = np.asarray(Wout, dtype=np.float32)
    bout = np.asarray(bout, dtype=np.float32)

    wout_bf = Wout.astype(ml_dtypes.bfloat16)
    bout_bf = bout.reshape(1, D).astype(ml_dtypes.bfloat16)
    _, _, perm_np, _ = _host_constants()
    ones_np = np.ones((128, 128), dtype=ml_dtypes.bfloat16)
    in_maps = []
    for c in range(8):
        b, g = c // 4, c % 4
        cols = slice(64 * HPC * g, 64 * HPC * (g + 1))  # this core's head dims
        # V weights augmented with a zero column per head slot; the matching
        # bias element is 1.0, so V tiles come out as [v(64) | 1] per head.
        wv_aug = np.zeros((D, 65 * HPC), dtype=np.float32)
        bv_aug = np.zeros((1, 65 * HPC), dtype=np.float32)
        wv_c = Wqkv[:, 2 * D :][:, cols]
        bv_c = bqkv[2 * D :][cols]
        for h in range(HPC):
            wv_aug[:, 65 * h : 65 * h + 64] = wv_c[:, 64 * h : 64 * h + 64]
            bv_aug[0, 65 * h : 65 * h + 64] = bv_c[64 * h : 64 * h + 64]
            bv_aug[0, 65 * h + 64] = 1.0
        in_maps.append(
            {
                "xb": np.ascontiguousarray(x[:, b, :].T).astype(ml_dtypes.bfloat16),
                "wq": np.ascontiguousarray(Wqkv[:, 0 * D :][:, cols]).astype(
                    ml_dtypes.bfloat16
                ),
                "wk": np.ascontiguousarray(Wqkv[:, 1 * D :][:, cols]).astype(
                    ml_dtypes.bfloat16
                ),
                "wv": wv_aug.astype(ml_dtypes.bfloat16),
                "bq": np.ascontiguousarray(bqkv[0 * D :][cols].reshape(NP, 128)),
                "bk": np.ascontiguousarray(bqkv[1 * D :][cols].reshape(NP, 128)),
                "bv": bv_aug.astype(ml_dtypes.bfloat16),
                "ones": ones_np,
                "perm": perm_np.astype(ml_dtypes.bfloat16),
                "wout": wout_bf,
                "bout": bout_bf,
            }
        )
    return in_maps


def unshard(results):
    out = np.empty((S, B, D), dtype=np.float32)
    for r in range(8):
        for b2 in range(B):
            out[SLC * r : SLC * (r + 1), b2, :] = results[r]["out_s"][b2]
    return out


def kernel(x, Wqkv, bqkv, Wout, bout, **_kw):
    nc = _get_program()
    in_maps = make_in_maps(x, Wqkv, bqkv, Wout, bout)
    res = run_bass_kernel_spmd(nc, in_maps, list(range(8)))
    return unshard(res.results)
